# revision 1
# baseline (speedup 1.0000x reference)
"""Trainium2 Bass kernel for causal multi-head attention (B=4, T=2048, C=1024, H=16).

Sharding: 8 NeuronCores = batch (4) x head-group (2). Each core computes, for
its batch b and its 8 heads:
  - QKV projections with column-sharded weights (Q^T/K^T in [D*,T] layout,
    V in [T, D*] layout),
  - causal attention with an appended validity/row-sum column on V
    (flash-style unnormalized accumulation + fused denominator),
  - row-sharded output projection producing a partial [T, C] output.
The host sums the two head-group partials per batch and adds the output bias.

All matmuls run in float32r (fp32 with 12-bit mantissa, full PE rate);
inputs that feed matmuls directly are pre-rounded to f32r on the host.

Schedule: one rolling loop — attention for query-block qb is emitted
interleaved with the projections of block qb+1, so the exp-bound Scalar-engine
stretches hide under PE-bound projection matmuls (Q^T rolls per block; y^T
spills through a DRAM bounce tile re-read by the output projection, which is
itself interleaved with the last block's attention via recycled tile tags).
Attention-V matmuls lag the score/exp stream (AV_LAG) to keep the in-order PE
queue from head-of-line blocking on the Scalar engine.
"""

import numpy as np
from contextlib import ExitStack

B, T, C, H = 4, 2048, 1024, 16
D = C // H            # 64
CL = C // 2           # 512 local channels (8 heads) per core
NCI = C // 128        # 8 contraction tiles for projections
PAIR_BLK = 192        # v_sb columns per head pair: [V_e(64) | valid(1) | gap(63) | V_o(64)]

_CACHE = {}

# schedule-pipelining knobs
AV_LAG = 3
ST_BUFS = 3
ES_BUFS = 5


def _f32r_round(a):
    """Round fp32 -> float32r bit pattern (keep top 12 mantissa bits, round half up)."""
    a = np.ascontiguousarray(a, dtype=np.float32)
    u = a.view(np.uint32).astype(np.uint64)
    r = ((u + 0x7FF + ((u >> 12) & 1)) & 0xFFFFF000).astype(np.uint32)
    return r.view(np.float32).reshape(a.shape)


def _build(t_len):
    import concourse.bass as bass  # noqa: F401
    import concourse.tile as tile
    from concourse import bacc, mybir

    dt = mybir.dt
    AF = mybir.ActivationFunctionType
    Alu = mybir.AluOpType

    NT = t_len // 128     # t tiles
    NB = t_len // 512     # t blocks

    nc = bacc.Bacc("TRN2", target_bir_lowering=False, debug=False,
                   enable_asserts=False, num_devices=8)

    xt_d = nc.dram_tensor("xt", (C, t_len), dt.float32r, kind="ExternalInput").ap()
    wq_d = nc.dram_tensor("wq", (C, CL), dt.float32r, kind="ExternalInput").ap()
    wk_d = nc.dram_tensor("wk", (C, CL), dt.float32r, kind="ExternalInput").ap()
    wv_d = nc.dram_tensor("wv", (C, CL), dt.float32r, kind="ExternalInput").ap()
    wp_d = nc.dram_tensor("wp", (CL, C), dt.float32r, kind="ExternalInput").ap()
    bq_d = nc.dram_tensor("bq", (CL, 1), dt.float32, kind="ExternalInput").ap()
    bk_d = nc.dram_tensor("bk", (CL, 1), dt.float32, kind="ExternalInput").ap()
    bvr_d = nc.dram_tensor("bvr", (1, CL), dt.float32r, kind="ExternalInput").ap()
    vm_d = nc.dram_tensor("vm", (128, NT), dt.float32, kind="ExternalInput").ap()
    mka_d = nc.dram_tensor("mka", (128, 256), dt.float32, kind="ExternalInput").ap()
    ones_d = nc.dram_tensor("ones", (128, 128), dt.float32r, kind="ExternalInput").ap()
    out_d = nc.dram_tensor("out", (t_len, C), dt.float32, kind="ExternalOutput").ap()

    with tile.TileContext(nc) as tc, ExitStack() as octx:
        persist = octx.enter_context(tc.tile_pool(name="persist", bufs=1))

        # Small persistent tensors
        maskadd = persist.tile([128, 256], dt.float32, tag="mka")
        nc.sync.dma_start(maskadd[:], mka_d[:])
        ones = persist.tile([128, 128], dt.float32r, tag="ones")
        nc.sync.dma_start(ones[:], ones_d[:])
        vm16 = persist.tile([128, NT], dt.float32, tag="vm16")
        nc.sync.dma_start(vm16[:], vm_d[:])
        bvr = persist.tile([1, CL], dt.float32r, tag="bvr")
        nc.sync.dma_start(bvr[:], bvr_d[:])
        bq_sb = persist.tile([128, 4], dt.float32, tag="bq")
        bk_sb = persist.tile([128, 4], dt.float32, tag="bk")
        for j in range(4):
            nc.sync.dma_start(bq_sb[:, j:j + 1], bq_d[j * 128:(j + 1) * 128, :])
            nc.sync.dma_start(bk_sb[:, j:j + 1], bk_d[j * 128:(j + 1) * 128, :])

        # Persistent activations (Q^T is rolled per t-block; K^T/V persist)
        kt_ = [persist.tile([128, t_len], dt.float32r, tag=f"kt{j}", name=f"kt{j}") for j in range(4)]
        vsb = [persist.tile([128, 4 * PAIR_BLK], dt.float32r, tag=f"v{t}",
                             name=f"v{t}") for t in range(NT)]

        # DRAM spill for y^T (read back by the output projection)
        dpool = octx.enter_context(tc.tile_pool(name="dram", bufs=1, space="DRAM"))
        ydram = dpool.tile([CL, t_len], dt.float32r, tag="yd", name="ydram")

        # ------- merged loop: projections for t-block tb, then attention qb=tb -------
        with (
            tc.tile_pool(name="pm", bufs=1) as pm,
            tc.tile_pool(name="psm", bufs=1, space="PSUM") as psm,
        ):
            wq_sb = [pm.tile([128, CL], dt.float32r, tag=f"wq{ci}", name=f"wq{ci}") for ci in range(NCI)]
            wk_sb = [pm.tile([128, CL], dt.float32r, tag=f"wk{ci}", name=f"wk{ci}") for ci in range(NCI)]
            wv_sb = [pm.tile([128, CL], dt.float32r, tag=f"wv{ci}", name=f"wv{ci}") for ci in range(NCI)]
            xs0 = []
            for ci in range(NCI):
                xti = pm.tile([128, 512], dt.float32r, tag=f"x{ci}", name=f"x0_{ci}")
                nc.sync.dma_start(xti[:], xt_d[ci * 128:(ci + 1) * 128, 0:512])
                nc.sync.dma_start(wq_sb[ci][:], wq_d[ci * 128:(ci + 1) * 128, :])
                xs0.append(xti)
            for ci in range(NCI):
                nc.sync.dma_start(wk_sb[ci][:], wk_d[ci * 128:(ci + 1) * 128, :])
                nc.sync.dma_start(wv_sb[ci][:], wv_d[ci * 128:(ci + 1) * 128, :])

            def emit_unit(qb, j, qtrj):
                q0 = qb * 512
                av0 = psm.tile([65, 512], dt.float32, tag="av0")
                av1 = psm.tile([128, 512], dt.float32, tag="av1")
                avs = (av0, av1)
                n_kt = qb * 4 + 4

                def emit_av(item):
                    h01, kt2, c02, width2, es2 = item
                    vofs = j * PAIR_BLK + h01 * 64
                    lw = 65 if h01 == 0 else 128
                    nc.tensor.matmul(
                        avs[h01][:, c02:512],
                        vsb[kt2][:, vofs:vofs + lw],
                        es2[:, 0:width2],
                        start=(kt2 == 0), stop=(kt2 == n_kt - 1))

                pend = []
                for kt in range(n_kt):
                    off = kt * 128 - q0
                    c0 = min(max(off, 0), 256)
                    width = 512 - c0
                    for h01 in range(2):
                        hb = h01 * 64
                        st = psm.tile([128, 512], dt.float32, tag="st",
                                      bufs=ST_BUFS)
                        nc.tensor.matmul(
                            st[:, 0:width],
                            kt_[j][hb:hb + 64, kt * 128:(kt + 1) * 128],
                            qtrj[hb:hb + 64, c0:512],
                            start=True, stop=True, tile_position=(hb, 0))
                        if off >= 0:
                            mw = off - c0 + 128
                            nc.vector.tensor_tensor(
                                st[:, 0:mw], st[:, 0:mw],
                                maskadd[:, 256 - mw:256], Alu.add)
                        es = pm.tile([128, 512], dt.float32r, tag=f"es{h01}",
                                     bufs=ES_BUFS)
                        nc.scalar.activation(es[:, 0:width], st[:, 0:width],
                                             AF.Exp, scale=0.125)
                        pend.append((h01, kt, c0, width, es))
                        while len(pend) > 2 * AV_LAG:
                            emit_av(pend.pop(0))
                for item in pend:
                    emit_av(item)
                # normalize: y^T = av_y * recip(broadcast(av_r)) -> DRAM spill
                sr = pm.tile([128, 512], dt.float32, tag="sr", bufs=1)
                nc.vector.tensor_copy(sr[64:65, :], av0[64:65, :])
                sr2 = pm.tile([1, 512], dt.float32, tag="sr2", bufs=2)
                nc.vector.tensor_copy(sr2[0:1, :], av1[0:1, :])
                ra = pm.tile([1, 512], dt.float32, tag="ra", bufs=2)
                nc.sync.dma_start(ra[0:1, :], sr[64:65, :])
                rra = pm.tile([1, 512], dt.float32, tag="rra", bufs=1)
                rrb = pm.tile([1, 512], dt.float32, tag="rrb", bufs=1)
                nc.vector.reciprocal_approx_fast(out=rra[0:1, :], in_=ra[0:1, :])
                nc.vector.reciprocal_approx_fast(out=rrb[0:1, :], in_=sr2[0:1, :])
                bca = pm.tile([128, 512], dt.float32, tag="bca", bufs=1)
                bcb = pm.tile([128, 512], dt.float32, tag="bcb", bufs=1)
                nc.gpsimd.partition_broadcast(bca[:, :], rra[0:1, :], channels=128)
                nc.gpsimd.partition_broadcast(bcb[:, :], rrb[0:1, :], channels=128)
                yst = pm.tile([128, 512], dt.float32r, tag="yst", bufs=3)
                nc.vector.tensor_mul(yst[0:64, :], av0[0:64, :], bca[0:64, :])
                nc.vector.tensor_mul(yst[64:128, :], av1[64:128, :],
                                     bcb[64:128, :])
                nc.sync.dma_start(ydram[j * 128:(j + 1) * 128, q0:q0 + 512],
                                  yst[:])

            def emit_proj(tts):
                for tt in tts:
                    yin = []
                    for j in range(4):
                        yj = pm.tile([128, 128], dt.float32r, tag=f"x{2 * j}",
                                     name=f"yin{j}")
                        nc.sync.dma_start(
                            yj[:],
                            ydram[j * 128:(j + 1) * 128, tt * 128:(tt + 1) * 128])
                        yin.append(yj)
                    for cb in range(2):
                        pj = psm.tile([128, 512], dt.float32, tag="vps", bufs=2)
                        for j in range(4):
                            wsl = wpa[j] if cb == 0 else wpb[j]
                            nc.tensor.matmul(pj[:], yin[j][:], wsl[:],
                                             start=(j == 0), stop=(j == 3))
                        po = pm.tile([128, 512], dt.float32, tag="yst", bufs=3)
                        nc.vector.tensor_copy(po[:], pj[:])
                        nc.sync.dma_start(
                            out_d[tt * 128:(tt + 1) * 128,
                                  cb * 512:(cb + 1) * 512],
                            po[:])


            wpa, wpb = [], []
            prev_qtr = None
            for tb in range(NB):
                ts = slice(tb * 512, (tb + 1) * 512)
                if tb == 0:
                    xs = xs0
                else:
                    xs = []
                    for ci in range(NCI):
                        xti = pm.tile([128, 512], dt.float32r, tag=f"x{ci}")
                        nc.sync.dma_start(xti[:], xt_d[ci * 128:(ci + 1) * 128, ts])
                        xs.append(xti)
                # Q^T (rolling, this block only) and K^T (persistent)
                qtr = []
                for j in range(4):
                    ps = psm.tile([128, 512], dt.float32, tag="qk", bufs=1)
                    for ci in range(NCI):
                        nc.tensor.matmul(
                            ps[:], wq_sb[ci][:, j * 128:(j + 1) * 128], xs[ci][:],
                            start=(ci == 0), stop=(ci == NCI - 1))
                    qj = pm.tile([128, 512], dt.float32r, tag=f"qtr{j}", name=f"qtr{j}", bufs=2)
                    nc.vector.tensor_scalar_add(qj[:], ps[:], bq_sb[:, j:j + 1])
                    qtr.append(qj)
                    if prev_qtr is not None:
                        emit_unit(tb - 1, j, prev_qtr[j])
                if tb == NB - 1:
                    for j in range(4):
                        wa = pm.tile([128, 512], dt.float32r, tag=f"wq{j}",
                                     name=f"wpa{j}")
                        nc.sync.dma_start(wa[:], wp_d[j * 128:(j + 1) * 128, 0:512])
                        wpa.append(wa)
                for j in range(4):
                    ps = psm.tile([128, 512], dt.float32, tag="qk", bufs=1)
                    for ci in range(NCI):
                        nc.tensor.matmul(
                            ps[:], wk_sb[ci][:, j * 128:(j + 1) * 128], xs[ci][:],
                            start=(ci == 0), stop=(ci == NCI - 1))
                    nc.vector.tensor_scalar_add(kt_[j][:, ts], ps[:], bk_sb[:, j:j + 1])
                if tb == NB - 1:
                    for j in range(4):
                        wb = pm.tile([128, 512], dt.float32r, tag=f"wk{j}",
                                     name=f"wpb{j}")
                        nc.sync.dma_start(wb[:], wp_d[j * 128:(j + 1) * 128, 512:1024])
                        wpb.append(wb)
                # V tiles for this block
                for tt in range(tb * 4, tb * 4 + 4):
                    lt = tt % 4
                    ps = psm.tile([128, CL], dt.float32, tag="vps", bufs=2)
                    for ci in range(NCI):
                        nc.tensor.matmul(
                            ps[:], xs[ci][:, lt * 128:(lt + 1) * 128], wv_sb[ci][:],
                            start=(ci == 0), stop=False)
                    nc.tensor.matmul(ps[:], ones[0:1, :], bvr[:],
                                     start=False, stop=True)
                    vt = vsb[tt]
                    vmc = vm16[:, tt:tt + 1]
                    ve_out = vt[:].rearrange("p (q b) -> p q b", b=PAIR_BLK)[:, :, 0:64]
                    ve_in = ps[:].rearrange("p (q b) -> p q b", b=128)[:, :, 0:64]
                    nc.vector.tensor_scalar_mul(ve_out, ve_in, vmc)
                    vo_out = vt[:].rearrange("p (q b) -> p q b", b=PAIR_BLK)[:, :, 128:192]
                    vo_in = ps[:].rearrange("p (q b) -> p q b", b=128)[:, :, 64:128]
                    nc.vector.tensor_scalar_mul(vo_out, vo_in, vmc)
                    for p_ in range(4):
                        nc.vector.tensor_copy(vt[:, p_ * PAIR_BLK + 64:p_ * PAIR_BLK + 65],
                                              vmc)
                    vg_out = vt[:].rearrange("p (q b) -> p q b", b=PAIR_BLK)[:, :, 65:128]
                    vg_in = ps[:].rearrange("p (q b) -> p q b", b=128)[:, :, 65:128]
                    nc.vector.tensor_scalar_mul(vg_out, vg_in, vmc)
                if tb == NB - 1 and NB > 1:
                    emit_proj(range(0, 3))
                prev_qtr = qtr

            # ---- tail: last-block attention interleaved with the projection ----
            # proj for blocks qb <= NB-2 interleaves with the tail units;
            # the last block's tiles go after its final unit
            done = (NB - 1) * 4  # ydram rows complete pre-tail (0..3 emitted in-loop)
            base = 3 if NB > 1 else 0
            for j in range(4):
                emit_unit(NB - 1, j, prev_qtr[j])
                if j < 3 and done > base:
                    lo = base + j * (done - base) // 3
                    hi = base + (j + 1) * (done - base) // 3
                    emit_proj(range(lo, hi))
            emit_proj(range(max(done, base) if NB > 1 else 0, NT))

    nc.compile()
    return nc


def _shard_inputs(x, attention_mask, Wq, bq, Wk, bk, Wv, bv, Wp, t_len):
    big = np.float32(-3.0e38)
    mka = np.full((128, 256), big, np.float32)
    r_, c_ = np.arange(128)[:, None], np.arange(128)[None, :]
    mka[:, 128:256] = np.where(c_ >= r_, np.float32(0.0), big)
    ones = _f32r_round(np.ones((128, 128), np.float32))
    in_maps = []
    for core in range(8):
        b, hg = core // 2, core % 2
        hs = slice(hg * CL, (hg + 1) * CL)
        in_maps.append({
            "xt": _f32r_round(x[b, :t_len].T),
            "wq": _f32r_round(Wq[:, hs]),
            "wk": _f32r_round(Wk[:, hs]),
            "wv": _f32r_round(Wv[:, hs]),
            "wp": _f32r_round(Wp[hs, :]),
            "bq": np.ascontiguousarray(bq[hs], np.float32).reshape(CL, 1),
            "bk": np.ascontiguousarray(bk[hs], np.float32).reshape(CL, 1),
            "bvr": _f32r_round(bv[hs].reshape(1, CL)),
            "vm": np.ascontiguousarray(
                attention_mask[b, :t_len].astype(np.float32).reshape(t_len // 128, 128).T),
            "mka": mka,
            "ones": ones,
        })
    return in_maps


def kernel(**inputs):
    from concourse import bass_utils

    t_len = T
    key = ("nc", t_len)
    if key not in _CACHE:
        _CACHE[key] = _build(t_len)
    nc = _CACHE[key]

    x = np.asarray(inputs["x"], dtype=np.float32)
    am = np.asarray(inputs["attention_mask"])
    in_maps = _shard_inputs(
        x, am, np.asarray(inputs["Wq"], np.float32), np.asarray(inputs["bq"], np.float32),
        np.asarray(inputs["Wk"], np.float32), np.asarray(inputs["bk"], np.float32),
        np.asarray(inputs["Wv"], np.float32), np.asarray(inputs["bv"], np.float32),
        np.asarray(inputs["Wp"], np.float32), t_len)

    res = bass_utils.run_bass_kernel_spmd(nc, in_maps, core_ids=list(range(8)))
    bp = np.asarray(inputs["bp"], np.float32)
    out = np.empty((B, T, C), dtype=np.float32)
    for b in range(B):
        out[b] = res.results[2 * b]["out"] + res.results[2 * b + 1]["out"] + bp
    return out



# revision 4
# speedup vs baseline: 61.3773x; 61.3773x over previous
"""Trainium2 Bass kernel for causal multi-head attention (B=4, T=2048, C=1024, H=16).

Sharding: 8 NeuronCores = batch (4) x head-group (2). Each core computes, for
its batch b and its 8 heads:
  - QKV projections with column-sharded weights (Q^T/K^T in [D*,T] layout,
    V in [T, D*] layout),
  - causal attention with an appended validity/row-sum column on V
    (flash-style unnormalized accumulation + fused denominator),
  - row-sharded output projection producing a partial [T, C] output.
The host sums the two head-group partials per batch and adds the output bias.

All matmuls run in float32r (fp32 with 12-bit mantissa, full PE rate);
inputs that feed matmuls directly are pre-rounded to f32r on the host.

Schedule: one rolling loop — attention for query-block qb is emitted
interleaved with the projections of block qb+1, so the exp-bound Scalar-engine
stretches hide under PE-bound projection matmuls (Q^T rolls per block; y^T
spills through a DRAM bounce tile re-read by the output projection, which is
itself interleaved with the last block's attention via recycled tile tags).
Attention-V matmuls lag the score/exp stream (AV_LAG) to keep the in-order PE
queue from head-of-line blocking on the Scalar engine.
"""

import numpy as np
from contextlib import ExitStack

B, T, C, H = 4, 2048, 1024, 16
D = C // H            # 64
CL = C // 2           # 512 local channels (8 heads) per core
NCI = C // 128        # 8 contraction tiles for projections
PAIR_BLK = 192        # v_sb columns per head pair: [V_e(64) | valid(1) | gap(63) | V_o(64)]

_CACHE = {}

# schedule-pipelining knobs
AV_LAG = 3
ST_BUFS = 3
ES_BUFS = 5


def _f32r_round(a):
    """Round fp32 -> float32r bit pattern (keep top 12 mantissa bits, round half up)."""
    a = np.ascontiguousarray(a, dtype=np.float32)
    u = a.view(np.uint32).astype(np.uint64)
    r = ((u + 0x7FF + ((u >> 12) & 1)) & 0xFFFFF000).astype(np.uint32)
    return r.view(np.float32).reshape(a.shape)


def _build(t_len, n_iter=1):
    import concourse.bass as bass  # noqa: F401
    import concourse.tile as tile
    from concourse import bacc, mybir

    dt = mybir.dt
    AF = mybir.ActivationFunctionType
    Alu = mybir.AluOpType

    NT = t_len // 128     # t tiles
    NB = t_len // 512     # t blocks

    nc = bacc.Bacc("TRN2", target_bir_lowering=False, debug=False,
                   enable_asserts=False, num_devices=8)

    xt_d = nc.dram_tensor("xt", (C, t_len), dt.float32r, kind="ExternalInput").ap()
    wq_d = nc.dram_tensor("wq", (C, CL), dt.float32r, kind="ExternalInput").ap()
    wk_d = nc.dram_tensor("wk", (C, CL), dt.float32r, kind="ExternalInput").ap()
    wv_d = nc.dram_tensor("wv", (C, CL), dt.float32r, kind="ExternalInput").ap()
    wp_d = nc.dram_tensor("wp", (CL, C), dt.float32r, kind="ExternalInput").ap()
    bq_d = nc.dram_tensor("bq", (CL, 1), dt.float32, kind="ExternalInput").ap()
    bk_d = nc.dram_tensor("bk", (CL, 1), dt.float32, kind="ExternalInput").ap()
    bvr_d = nc.dram_tensor("bvr", (1, CL), dt.float32r, kind="ExternalInput").ap()
    vm_d = nc.dram_tensor("vm", (128, NT), dt.float32, kind="ExternalInput").ap()
    mka_d = nc.dram_tensor("mka", (128, 256), dt.float32, kind="ExternalInput").ap()
    ones_d = nc.dram_tensor("ones", (128, 128), dt.float32r, kind="ExternalInput").ap()
    out_d = nc.dram_tensor("out", (t_len, C), dt.float32, kind="ExternalOutput").ap()

    with tile.TileContext(nc) as tc:
        for _it in range(n_iter):
            _emit_iter(nc, tc, tile, dt, AF, Alu, t_len, NT, NB,
                       xt_d, wq_d, wk_d, wv_d, wp_d, bq_d, bk_d, bvr_d,
                       vm_d, mka_d, ones_d, out_d)

    nc.compile()
    return nc


def _emit_iter(nc, tc, tile, dt, AF, Alu, t_len, NT, NB,
               xt_d, wq_d, wk_d, wv_d, wp_d, bq_d, bk_d, bvr_d,
               vm_d, mka_d, ones_d, out_d):
    with ExitStack() as octx:
        persist = octx.enter_context(tc.tile_pool(name="persist", bufs=1))

        # Small persistent tensors
        maskadd = persist.tile([128, 256], dt.float32, tag="mka")
        nc.sync.dma_start(maskadd[:], mka_d[:])
        ones = persist.tile([128, 128], dt.float32r, tag="ones")
        nc.sync.dma_start(ones[:], ones_d[:])
        vm16 = persist.tile([128, NT], dt.float32, tag="vm16")
        nc.sync.dma_start(vm16[:], vm_d[:])
        bvr = persist.tile([1, CL], dt.float32r, tag="bvr")
        nc.sync.dma_start(bvr[:], bvr_d[:])
        bq_sb = persist.tile([128, 4], dt.float32, tag="bq")
        bk_sb = persist.tile([128, 4], dt.float32, tag="bk")
        for j in range(4):
            nc.sync.dma_start(bq_sb[:, j:j + 1], bq_d[j * 128:(j + 1) * 128, :])
            nc.sync.dma_start(bk_sb[:, j:j + 1], bk_d[j * 128:(j + 1) * 128, :])

        # Persistent activations (Q^T is rolled per t-block; K^T/V persist)
        kt_ = [persist.tile([128, t_len], dt.float32r, tag=f"kt{j}", name=f"kt{j}") for j in range(4)]
        vsb = [persist.tile([128, 4 * PAIR_BLK], dt.float32r, tag=f"v{t}",
                             name=f"v{t}") for t in range(NT)]

        # DRAM spill for y^T (read back by the output projection)
        dpool = octx.enter_context(tc.tile_pool(name="dram", bufs=1, space="DRAM"))
        ydram = dpool.tile([CL, t_len], dt.float32r, tag="yd", name="ydram")

        # ------- merged loop: projections for t-block tb, then attention qb=tb -------
        with (
            tc.tile_pool(name="pm", bufs=1) as pm,
            tc.tile_pool(name="psm", bufs=1, space="PSUM") as psm,
        ):
            wq_sb = [pm.tile([128, CL], dt.float32r, tag=f"wq{ci}", name=f"wq{ci}") for ci in range(NCI)]
            wk_sb = [pm.tile([128, CL], dt.float32r, tag=f"wk{ci}", name=f"wk{ci}") for ci in range(NCI)]
            wv_sb = [pm.tile([128, CL], dt.float32r, tag=f"wv{ci}", name=f"wv{ci}") for ci in range(NCI)]
            xs0 = []
            for ci in range(NCI):
                xti = pm.tile([128, 512], dt.float32r, tag=f"x{ci}", name=f"x0_{ci}")
                nc.sync.dma_start(xti[:], xt_d[ci * 128:(ci + 1) * 128, 0:512])
                nc.sync.dma_start(wq_sb[ci][:], wq_d[ci * 128:(ci + 1) * 128, :])
                xs0.append(xti)
            for ci in range(NCI):
                nc.sync.dma_start(wk_sb[ci][:], wk_d[ci * 128:(ci + 1) * 128, :])
                nc.sync.dma_start(wv_sb[ci][:], wv_d[ci * 128:(ci + 1) * 128, :])

            def emit_unit(qb, j, qtrj):
                q0 = qb * 512
                av0 = psm.tile([65, 512], dt.float32, tag="av0")
                av1 = psm.tile([128, 512], dt.float32, tag="av1")
                avs = (av0, av1)
                n_kt = qb * 4 + 4

                def emit_av(item):
                    h01, kt2, c02, width2, es2 = item
                    vofs = j * PAIR_BLK + h01 * 64
                    lw = 65 if h01 == 0 else 128
                    nc.tensor.matmul(
                        avs[h01][:, c02:512],
                        vsb[kt2][:, vofs:vofs + lw],
                        es2[:, 0:width2],
                        start=(kt2 == 0), stop=(kt2 == n_kt - 1))

                pend = []
                for kt in range(n_kt):
                    off = kt * 128 - q0
                    c0 = min(max(off, 0), 256)
                    width = 512 - c0
                    for h01 in range(2):
                        hb = h01 * 64
                        st = psm.tile([128, 512], dt.float32, tag="st",
                                      bufs=ST_BUFS)
                        nc.tensor.matmul(
                            st[:, 0:width],
                            kt_[j][hb:hb + 64, kt * 128:(kt + 1) * 128],
                            qtrj[hb:hb + 64, c0:512],
                            start=True, stop=True, tile_position=(hb, 0))
                        if off >= 0:
                            mw = off - c0 + 128
                            nc.vector.tensor_tensor(
                                st[:, 0:mw], st[:, 0:mw],
                                maskadd[:, 256 - mw:256], Alu.add)
                        es = pm.tile([128, 512], dt.float32r, tag=f"es{h01}",
                                     bufs=ES_BUFS)
                        nc.scalar.activation(es[:, 0:width], st[:, 0:width],
                                             AF.Exp, scale=0.125)
                        pend.append((h01, kt, c0, width, es))
                        while len(pend) > 2 * AV_LAG:
                            emit_av(pend.pop(0))
                for item in pend:
                    emit_av(item)
                # normalize: y^T = av_y * recip(broadcast(av_r)) -> DRAM spill
                sr = pm.tile([128, 512], dt.float32, tag="sr", bufs=1)
                nc.vector.tensor_copy(sr[64:65, :], av0[64:65, :])
                sr2 = pm.tile([1, 512], dt.float32, tag="sr2", bufs=2)
                nc.vector.tensor_copy(sr2[0:1, :], av1[0:1, :])
                ra = pm.tile([1, 512], dt.float32, tag="ra", bufs=2)
                nc.sync.dma_start(ra[0:1, :], sr[64:65, :])
                rra = pm.tile([1, 512], dt.float32, tag="rra", bufs=1)
                rrb = pm.tile([1, 512], dt.float32, tag="rrb", bufs=1)
                nc.vector.reciprocal_approx_fast(out=rra[0:1, :], in_=ra[0:1, :])
                nc.vector.reciprocal_approx_fast(out=rrb[0:1, :], in_=sr2[0:1, :])
                bca = pm.tile([128, 512], dt.float32, tag="bca", bufs=1)
                bcb = pm.tile([128, 512], dt.float32, tag="bcb", bufs=1)
                nc.gpsimd.partition_broadcast(bca[:, :], rra[0:1, :], channels=128)
                nc.gpsimd.partition_broadcast(bcb[:, :], rrb[0:1, :], channels=128)
                yst = pm.tile([128, 512], dt.float32r, tag="yst", bufs=3)
                nc.vector.tensor_mul(yst[0:64, :], av0[0:64, :], bca[0:64, :])
                nc.vector.tensor_mul(yst[64:128, :], av1[64:128, :],
                                     bcb[64:128, :])
                nc.sync.dma_start(ydram[j * 128:(j + 1) * 128, q0:q0 + 512],
                                  yst[:])

            def emit_proj(tts):
                for tt in tts:
                    yin = []
                    for j in range(4):
                        yj = pm.tile([128, 128], dt.float32r, tag=f"x{2 * j}",
                                     name=f"yin{j}")
                        nc.sync.dma_start(
                            yj[:],
                            ydram[j * 128:(j + 1) * 128, tt * 128:(tt + 1) * 128])
                        yin.append(yj)
                    for cb in range(2):
                        pj = psm.tile([128, 512], dt.float32, tag="vps", bufs=2)
                        for j in range(4):
                            wsl = wpa[j] if cb == 0 else wpb[j]
                            nc.tensor.matmul(pj[:], yin[j][:], wsl[:],
                                             start=(j == 0), stop=(j == 3))
                        po = pm.tile([128, 512], dt.float32, tag="yst", bufs=3)
                        nc.vector.tensor_copy(po[:], pj[:])
                        nc.sync.dma_start(
                            out_d[tt * 128:(tt + 1) * 128,
                                  cb * 512:(cb + 1) * 512],
                            po[:])


            wpa, wpb = [], []
            prev_qtr = None
            for tb in range(NB):
                ts = slice(tb * 512, (tb + 1) * 512)
                if tb == 0:
                    xs = xs0
                else:
                    xs = []
                    for ci in range(NCI):
                        xti = pm.tile([128, 512], dt.float32r, tag=f"x{ci}")
                        nc.sync.dma_start(xti[:], xt_d[ci * 128:(ci + 1) * 128, ts])
                        xs.append(xti)
                # Q^T (rolling, this block only) and K^T (persistent)
                qtr = []
                for j in range(4):
                    ps = psm.tile([128, 512], dt.float32, tag="qk", bufs=1)
                    for ci in range(NCI):
                        nc.tensor.matmul(
                            ps[:], wq_sb[ci][:, j * 128:(j + 1) * 128], xs[ci][:],
                            start=(ci == 0), stop=(ci == NCI - 1))
                    qj = pm.tile([128, 512], dt.float32r, tag=f"qtr{j}", name=f"qtr{j}", bufs=2)
                    nc.vector.tensor_scalar_add(qj[:], ps[:], bq_sb[:, j:j + 1])
                    qtr.append(qj)
                    if prev_qtr is not None:
                        emit_unit(tb - 1, j, prev_qtr[j])
                if tb == NB - 1:
                    for j in range(4):
                        wa = pm.tile([128, 512], dt.float32r, tag=f"wq{j}",
                                     name=f"wpa{j}")
                        nc.sync.dma_start(wa[:], wp_d[j * 128:(j + 1) * 128, 0:512])
                        wpa.append(wa)
                for j in range(4):
                    ps = psm.tile([128, 512], dt.float32, tag="qk", bufs=1)
                    for ci in range(NCI):
                        nc.tensor.matmul(
                            ps[:], wk_sb[ci][:, j * 128:(j + 1) * 128], xs[ci][:],
                            start=(ci == 0), stop=(ci == NCI - 1))
                    nc.vector.tensor_scalar_add(kt_[j][:, ts], ps[:], bk_sb[:, j:j + 1])
                if tb == NB - 1:
                    for j in range(4):
                        wb = pm.tile([128, 512], dt.float32r, tag=f"wk{j}",
                                     name=f"wpb{j}")
                        nc.sync.dma_start(wb[:], wp_d[j * 128:(j + 1) * 128, 512:1024])
                        wpb.append(wb)
                # V tiles for this block
                for tt in range(tb * 4, tb * 4 + 4):
                    lt = tt % 4
                    ps = psm.tile([128, CL], dt.float32, tag="vps", bufs=2)
                    for ci in range(NCI):
                        nc.tensor.matmul(
                            ps[:], xs[ci][:, lt * 128:(lt + 1) * 128], wv_sb[ci][:],
                            start=(ci == 0), stop=False)
                    nc.tensor.matmul(ps[:], ones[0:1, :], bvr[:],
                                     start=False, stop=True)
                    vt = vsb[tt]
                    vmc = vm16[:, tt:tt + 1]
                    ve_out = vt[:].rearrange("p (q b) -> p q b", b=PAIR_BLK)[:, :, 0:64]
                    ve_in = ps[:].rearrange("p (q b) -> p q b", b=128)[:, :, 0:64]
                    nc.vector.tensor_scalar_mul(ve_out, ve_in, vmc)
                    vo_out = vt[:].rearrange("p (q b) -> p q b", b=PAIR_BLK)[:, :, 128:192]
                    vo_in = ps[:].rearrange("p (q b) -> p q b", b=128)[:, :, 64:128]
                    nc.vector.tensor_scalar_mul(vo_out, vo_in, vmc)
                    for p_ in range(4):
                        nc.vector.tensor_copy(vt[:, p_ * PAIR_BLK + 64:p_ * PAIR_BLK + 65],
                                              vmc)
                    vg_out = vt[:].rearrange("p (q b) -> p q b", b=PAIR_BLK)[:, :, 65:128]
                    vg_in = ps[:].rearrange("p (q b) -> p q b", b=128)[:, :, 65:128]
                    nc.vector.tensor_scalar_mul(vg_out, vg_in, vmc)
                if tb == NB - 1 and NB > 1:
                    emit_proj(range(0, 3))
                prev_qtr = qtr

            # ---- tail: last-block attention interleaved with the projection ----
            # proj for blocks qb <= NB-2 interleaves with the tail units;
            # the last block's tiles go after its final unit
            done = (NB - 1) * 4  # ydram rows complete pre-tail (0..3 emitted in-loop)
            base = 3 if NB > 1 else 0
            for j in range(4):
                emit_unit(NB - 1, j, prev_qtr[j])
                if j < 3 and done > base:
                    lo = base + j * (done - base) // 3
                    hi = base + (j + 1) * (done - base) // 3
                    emit_proj(range(lo, hi))
            emit_proj(range(max(done, base) if NB > 1 else 0, NT))


def _shard_inputs(x, attention_mask, Wq, bq, Wk, bk, Wv, bv, Wp, t_len):
    big = np.float32(-3.0e38)
    mka = np.full((128, 256), big, np.float32)
    r_, c_ = np.arange(128)[:, None], np.arange(128)[None, :]
    mka[:, 128:256] = np.where(c_ >= r_, np.float32(0.0), big)
    ones = _f32r_round(np.ones((128, 128), np.float32))
    in_maps = []
    for core in range(8):
        b, hg = core // 2, core % 2
        hs = slice(hg * CL, (hg + 1) * CL)
        in_maps.append({
            "xt": _f32r_round(x[b, :t_len].T),
            "wq": _f32r_round(Wq[:, hs]),
            "wk": _f32r_round(Wk[:, hs]),
            "wv": _f32r_round(Wv[:, hs]),
            "wp": _f32r_round(Wp[hs, :]),
            "bq": np.ascontiguousarray(bq[hs], np.float32).reshape(CL, 1),
            "bk": np.ascontiguousarray(bk[hs], np.float32).reshape(CL, 1),
            "bvr": _f32r_round(bv[hs].reshape(1, CL)),
            "vm": np.ascontiguousarray(
                attention_mask[b, :t_len].astype(np.float32).reshape(t_len // 128, 128).T),
            "mka": mka,
            "ones": ones,
        })
    return in_maps


def kernel(**inputs):
    from concourse import bass_utils

    t_len = T
    key = ("nc", t_len)
    if key not in _CACHE:
        _CACHE[key] = _build(t_len)
    nc = _CACHE[key]

    x = np.asarray(inputs["x"], dtype=np.float32)
    am = np.asarray(inputs["attention_mask"])
    in_maps = _shard_inputs(
        x, am, np.asarray(inputs["Wq"], np.float32), np.asarray(inputs["bq"], np.float32),
        np.asarray(inputs["Wk"], np.float32), np.asarray(inputs["bk"], np.float32),
        np.asarray(inputs["Wv"], np.float32), np.asarray(inputs["bv"], np.float32),
        np.asarray(inputs["Wp"], np.float32), t_len)

    res = bass_utils.run_bass_kernel_spmd(nc, in_maps, core_ids=list(range(8)))
    bp = np.asarray(inputs["bp"], np.float32)
    out = np.empty((B, T, C), dtype=np.float32)
    for b in range(B):
        out[b] = res.results[2 * b]["out"] + res.results[2 * b + 1]["out"] + bp
    return out

